# revision 56
# baseline (speedup 1.0000x reference)
"""Multi-head attention (B=4, S=2048, D=1024, H=16, causal) on 8 TRN2 cores.

Sharding: core c -> (batch b = c//2, head-group g = c%2 of 8 heads).
Each core computes projections for its 8 heads (column-split Wq/Wk/Wv),
flash-style causal attention, and a partial output projection (row-split Wo).
Host unshard sums the two partials per batch and adds bo.

v2 (all-bf16, fully software-pipelined single pass):
- Everything bf16 on the PE (error budget allows: score errors shrink 8x
  under the 1/sqrt(HD) softmax scale). Halves DMA + SBUF, removes the
  fp32r N<256 4x-cycle penalty, enables FWL weight loads.
- qc-outer schedule: per query chunk, all head pairs run scores->exp->PV
  with a 2-step lag (PV of j-2 issues after scores of j), and projection /
  output-projection matmuls are issued between attention steps as PE
  filler so the PE never idles and HAM stays at 2.4 GHz.
- A+B exp fused into one [128,1024] ACTIVATE per j-block.
- Attention output is written back into the qwT tile (queries for chunk qc
  are dead after their scores), saving 16KB/partition of SBUF.
- Denominators ride the PV matmul as a per-head ones column (M=65).
"""

from collections import deque
from contextlib import ExitStack

import ml_dtypes
import numpy as np

import concourse.bass as bass
import concourse.tile as tile
from concourse import bacc, mybir
from concourse.bass_utils import run_bass_kernel_spmd

F32 = mybir.dt.float32
F16 = mybir.dt.float16
BF = mybir.dt.bfloat16
EXP = mybir.ActivationFunctionType.Exp
COPY = mybir.ActivationFunctionType.Copy
BF_NP = ml_dtypes.bfloat16

B, S, D, H = 4, 2048, 1024, 16
HD = D // H          # 64
DL = D // 2          # 512 local douts per core
NT = DL // 128       # 4 dout tiles / head pairs
NR = S // 128        # 16 key row tiles
NQ = S // 512        # 4 query chunks
NDIN = D // 128      # 8 din tiles
LAG = 3              # scores(j) -> PV(j-LAG) software pipeline depth


def build_nc():
    nc = bacc.Bacc("TRN2", target_bir_lowering=False, debug=False, num_devices=8)

    qT = nc.dram_tensor("qT", [D, S], BF, kind="ExternalInput").ap()
    kT = nc.dram_tensor("kT", [D, S], BF, kind="ExternalInput").ap()
    vT = nc.dram_tensor("vT", [D, S], BF, kind="ExternalInput").ap()
    Wq_s = nc.dram_tensor("Wq_s", [D, DL], BF, kind="ExternalInput").ap()
    Wk_s = nc.dram_tensor("Wk_s", [D, DL], BF, kind="ExternalInput").ap()
    Wv_s = nc.dram_tensor("Wv_s", [D, DL], BF, kind="ExternalInput").ap()
    Wo_s = nc.dram_tensor("Wo_s", [DL, D], BF, kind="ExternalInput").ap()
    bq_s = nc.dram_tensor("bq_s", [DL, 1], F32, kind="ExternalInput").ap()
    bk_s = nc.dram_tensor("bk_s", [DL, 1], F32, kind="ExternalInput").ap()
    bv_bc = nc.dram_tensor("bv_bc", [128, DL], F32, kind="ExternalInput").ap()
    E_in = nc.dram_tensor("E_in", [8, DL], BF, kind="ExternalInput").ap()
    ident_in = nc.dram_tensor("ident_in", [128, 128], BF, kind="ExternalInput").ap()
    maskb_in = nc.dram_tensor("maskb_in", [128, 128], BF, kind="ExternalInput").ap()
    out_p = nc.dram_tensor("out_partial", [S, D], F16, kind="ExternalOutput").ap()

    with tile.TileContext(nc) as tc, ExitStack() as ctx:
        keep = ctx.enter_context(tc.tile_pool(name="keep", bufs=1))
        qslp = ctx.enter_context(tc.tile_pool(name="qsl", bufs=2))
        kslp = ctx.enter_context(tc.tile_pool(name="ksl", bufs=2))
        vslp = ctx.enter_context(tc.tile_pool(name="vsl", bufs=2))
        wtp = ctx.enter_context(tc.tile_pool(name="wt", bufs=1))
        prp = ctx.enter_context(tc.tile_pool(name="probs", bufs=1))
        stgp = ctx.enter_context(tc.tile_pool(name="stg", bufs=2))
        osp = ctx.enter_context(tc.tile_pool(name="osb", bufs=3))
        scp = ctx.enter_context(tc.tile_pool(name="scps", bufs=2, space="PSUM"))
        atp = ctx.enter_context(tc.tile_pool(name="atps", bufs=1, space="PSUM"))
        mip = ctx.enter_context(tc.tile_pool(name="mips", bufs=2, space="PSUM"))

        # ---------------- persistent SBUF ----------------
        # hw[t]: Q^T for head pair t during scores, then overwritten per qc
        # chunk with the (unnormalized) attention output of pair t.
        hw = [keep.tile([128, S], BF, tag=f"hw{t}", name=f"hw{t}") for t in range(NT)]
        kwT = [keep.tile([128, S], BF, tag=f"kwT{t}", name=f"kwT{t}") for t in range(NT)]
        vw = [keep.tile([128, 8 * 65], BF, tag=f"vw{r}", name=f"vw{r}") for r in range(NR)]
        sums = keep.tile([8, S], F32, tag="sums")
        recip = keep.tile([8, S], BF, tag="recip")
        bias_q = keep.tile([128, NT], F32, tag="bias_q")
        bias_k = keep.tile([128, NT], F32, tag="bias_k")
        bv_sb = keep.tile([128, DL], F32, tag="bv_sb")
        E_sb = keep.tile([8, DL], BF, tag="E_sb")
        ident = keep.tile([128, 128], BF, tag="ident")
        maskb = keep.tile([128, 128], BF, tag="maskb")

        # never-written recip rows are multiplied by E's zeros in the
        # broadcast matmul; they must not hold NaN garbage.
        nc.vector.memset(recip[:], 1.0)

        # small/constant loads, spread off the startup critical streams
        for t in range(NT):
            nc.sync.dma_start(bias_q[:, t:t + 1], bq_s[128 * t:128 * (t + 1), :])
            nc.sync.dma_start(bias_k[:, t:t + 1], bk_s[128 * t:128 * (t + 1), :])
        nc.scalar.dma_start(bv_sb[:], bv_bc)
        nc.scalar.dma_start(E_sb[:], E_in)
        nc.scalar.dma_start(ident[:], ident_in)
        nc.scalar.dma_start(maskb[:], maskb_in)

        # per-head ones column in vw (PV emits softmax denominators for free)
        for r in range(NR):
            ones_ap = vw[r][:].rearrange("p (h e) -> p h e", e=65)[:, :, 64:65]
            nc.gpsimd.memset(ones_ap, 1.0)

        # projection weight tiles — DMAs are issued interleaved with the
        # first slab chunks in the schedule section (per-dn pipelining)
        wq_sb = [wtp.tile([128, DL], BF, tag=f"wq{dn}", name=f"wq{dn}")
                 for dn in range(NDIN)]
        wk_sb = [wtp.tile([128, DL], BF, tag=f"wk{dn}", name=f"wk{dn}")
                 for dn in range(NDIN)]
        wv_sb = [wtp.tile([128, DL], BF, tag=f"wv{dn}", name=f"wv{dn}")
                 for dn in range(NDIN)]
        wo_sb = [wtp.tile([128, D], BF, tag=f"wo{t}", name=f"wo{t}")
                 for t in range(NT)]

        # ---------------- chunked slab loads ----------------
        qsl = {}  # qsl[qc][dn] -> [128, 512] bf16 tile of qT
        ksl = {}
        vsl = {}

        def load_chunk(store, pool, src, c, pfx, eng):
            tiles = []
            for dn in range(NDIN):
                t_ = pool.tile([128, 512], BF, tag=f"{pfx}{dn}")
                eng.dma_start(t_[:], src[128 * dn:128 * (dn + 1), 512 * c:512 * (c + 1)])
                tiles.append(t_)
            store[c] = tiles

        # ---------------- filler generators ----------------
        proj_gens = deque()
        out_gens = deque()
        pending_norms = []

        def fill(n):
            done = 0
            while done < n:
                q = proj_gens if proj_gens else out_gens
                if not q:
                    return
                try:
                    next(q[0])
                    done += 1
                except StopIteration:
                    q.popleft()

        def drain_proj():
            while proj_gens:
                try:
                    next(proj_gens[0])
                except StopIteration:
                    proj_gens.popleft()

        def drain_all():
            drain_proj()
            while out_gens:
                try:
                    next(out_gens[0])
                except StopIteration:
                    out_gens.popleft()

        def projQ_gen(qc):
            for t in range(NT):
                ps = mip.tile([128, 512], F32, tag="mx")
                for dn in range(NDIN):
                    if dn % 2 == 0 and dn > 0:
                        yield
                    nc.tensor.matmul(
                        ps[:], wq_sb[dn][:, 128 * t:128 * (t + 1)], qsl[qc][dn][:],
                        start=(dn == 0), stop=(dn == NDIN - 1))
                nc.vector.tensor_scalar_add(
                    hw[t][:, 512 * qc:512 * (qc + 1)], ps[:], bias_q[:, t:t + 1])
                yield

        def projK_gen(rc):
            for t in range(NT):
                ps = mip.tile([128, 512], F32, tag="mx")
                for dn in range(NDIN):
                    if dn % 2 == 0 and dn > 0:
                        yield
                    nc.tensor.matmul(
                        ps[:], wk_sb[dn][:, 128 * t:128 * (t + 1)], ksl[rc][dn][:],
                        start=(dn == 0), stop=(dn == NDIN - 1))
                nc.vector.tensor_scalar_add(
                    kwT[t][:, 512 * rc:512 * (rc + 1)], ps[:], bias_k[:, t:t + 1])
                yield

        def projV_gen(g):
            for r in range(4 * g, 4 * g + 4):
                ps = mip.tile([128, 512], F32, tag="mx")
                for dn in range(NDIN):
                    if dn % 2 == 0 and dn > 0:
                        yield
                    nc.tensor.matmul(
                        ps[:], vsl[g][dn][:, 128 * (r - 4 * g):128 * (r - 4 * g + 1)],
                        wv_sb[dn][:],
                        start=(dn == 0), stop=(dn == NDIN - 1))
                dst3 = vw[r][:].rearrange("p (h e) -> p h e", e=65)[:, :, 0:64]
                nc.vector.tensor_add(
                    dst3, ps[:].rearrange("p (h e) -> p h e", e=64),
                    bv_sb[:].rearrange("p (h e) -> p h e", e=64))
                yield

        def norm_gen(p, qc):
            # broadcast 1/denominator over the pair's 128 dims and rescale
            qf = slice(512 * qc, 512 * (qc + 1))
            bc = mip.tile([128, 512], F32, tag="mx")
            nc.tensor.matmul(bc[:], E_sb[:, 128 * p:128 * (p + 1)],
                             recip[:, qf], start=True, stop=True)
            nc.vector.tensor_mul(hw[p][:, qf], hw[p][:, qf], bc[:])
            yield

        def outproj_gen(qc):
            for rt in range(4 * qc, 4 * qc + 4):
                for nch in range(2):
                    po = mip.tile([128, 512], F32, tag="mx")
                    for t in range(NT):
                        nc.tensor.matmul(
                            po[:], hw[t][:, 128 * rt:128 * (rt + 1)],
                            wo_sb[t][:, 512 * nch:512 * (nch + 1)],
                            start=(t == 0), stop=(t == NT - 1))
                        if t % 2 == 1:
                            yield
                    ob = osp.tile([128, 512], F16, tag="ob")
                    nc.vector.tensor_copy(ob[:], po[:])
                    eng = nc.scalar if rt % 2 else nc.sync
                    eng.dma_start(
                        out_p[128 * rt:128 * (rt + 1), 512 * nch:512 * (nch + 1)], ob[:])

        # ---------------- attention ----------------
        def attention(p, qc):
            jmax = 4 * qc + 3
            atA = atp.tile([65, 512], F32, tag="atA")
            atB = atp.tile([65, 512], F32, tag="atB")
            pend = {}
            for step in range(jmax + 1 + LAG):
                if step <= jmax:
                    j = step
                    off = max(0, 128 * j - 512 * qc)
                    diag = 128 * j >= 512 * qc
                    qs = slice(512 * qc + off, 512 * (qc + 1))
                    sAB = scp.tile([128, 1024], F32, tag="sAB")
                    nc.tensor.matmul(
                        sAB[:, off:512],
                        kwT[p][0:64, 128 * j:128 * (j + 1)], hw[p][0:64, qs],
                        start=True, stop=True, tile_position=(0, 0))
                    nc.tensor.matmul(
                        sAB[:, 512 + off:1024],
                        kwT[p][64:128, 128 * j:128 * (j + 1)], hw[p][64:128, qs],
                        start=True, stop=True, tile_position=(64, 0))
                    pAB = prp.tile([128, 1024], BF, tag=f"p{j % 6}")
                    if off == 0:
                        nc.scalar.activation(pAB[:], sAB[:], EXP, scale=0.125)
                    else:
                        # one strided ACTIVATE covers both halves' live columns
                        s3 = sAB[:].rearrange("p (b c) -> p b c", b=2)[:, :, off:512]
                        p3 = pAB[:].rearrange("p (b c) -> p b c", b=2)[:, :, off:512]
                        nc.scalar.activation(p3, s3, EXP, scale=0.125)
                    if diag:  # causal mask on the diagonal 128-col strip
                        for cb in (off, 512 + off):
                            nc.gpsimd.affine_select(
                                out=pAB[:, cb:cb + 128], in_=pAB[:, cb:cb + 128],
                                channel_multiplier=-1, pattern=[[1, 128]], base=0,
                                compare_op=mybir.AluOpType.is_ge, fill=0.0)
                    pend[j] = (pAB, off)
                jj = step - LAG
                if 0 <= jj:
                    pAB, off = pend.pop(jj)
                    nc.tensor.matmul(
                        atA[0:65, off:512],
                        vw[jj][:, 65 * 2 * p:65 * 2 * p + 65], pAB[:, off:512],
                        start=(jj == 0), stop=(jj == jmax))
                    nc.tensor.matmul(
                        atB[0:65, off:512],
                        vw[jj][:, 65 * (2 * p + 1):65 * (2 * p + 1) + 65],
                        pAB[:, 512 + off:1024],
                        start=(jj == 0), stop=(jj == jmax))
                if step <= jmax:
                    # no fills during the PV drain steps: their DVE consumers
                    # would queue ahead of the epilogue copies below
                    fill(1)
            # epilogue: write attention output over the dead Q columns and
            # stage the denominators (PSUM row 64) out to sums.
            qf = slice(512 * qc, 512 * (qc + 1))
            nc.vector.tensor_copy(hw[p][0:64, qf], atA[0:64, :])
            nc.vector.tensor_copy(hw[p][64:128, qf], atB[0:64, :])
            stgA = stgp.tile([1, 512], F32, tag="stgA")
            stgB = stgp.tile([1, 512], F32, tag="stgB")
            nc.vector.tensor_copy(stgA[:], atA[64:65, :])
            nc.vector.tensor_copy(stgB[:], atB[64:65, :])
            nc.sync.dma_start(sums[2 * p:2 * p + 1, qf], stgA[:])
            nc.sync.dma_start(sums[2 * p + 1:2 * p + 2, qf], stgB[:])
            fill(LAG)

        # ---------------- schedule ----------------
        # startup streams, balanced for ~80GB/s HWDGE queues and the
        # ~170GB/s gpsimd SWDGE bulk path:
        #   gpsimd: wq, wk, wv   sync: q0 + half v0   scalar: k0 + half v0, wo
        for dn in range(NDIN):
            nc.gpsimd.dma_start(wq_sb[dn][:], Wq_s[128 * dn:128 * (dn + 1), :])
        for dn in range(NDIN):
            nc.gpsimd.dma_start(wk_sb[dn][:], Wk_s[128 * dn:128 * (dn + 1), :])
        for dn in range(NDIN):
            nc.gpsimd.dma_start(wv_sb[dn][:], Wv_s[128 * dn:128 * (dn + 1), :])
        q0t, k0t, v0t = [], [], []
        for src, store, pool, pfx in ((qT, q0t, qslp, "q"), (kT, k0t, kslp, "k"),
                                      (vT, v0t, vslp, "v")):
            for dn in range(NDIN):
                t_ = pool.tile([128, 512], BF, tag=f"{pfx}{dn}", name=f"{pfx}0_{dn}")
                eng = nc.sync if dn % 2 == 0 else nc.scalar
                eng.dma_start(t_[:], src[128 * dn:128 * (dn + 1), 0:512])
                store.append(t_)
        qsl[0], ksl[0], vsl[0] = q0t, k0t, v0t
        for t in range(NT):
            nc.scalar.dma_start(wo_sb[t][:], Wo_s[128 * t:128 * (t + 1), :])

        # startup: project pair p's Q/K (and all of V before the first PV),
        # starting attention(p, 0) as soon as its own tiles are ready
        gq, gk, gv = projQ_gen(0), projK_gen(0), projV_gen(0)

        def advance(g, n):
            for _ in range(n):
                try:
                    next(g)
                except StopIteration:
                    return

        for qc in range(NQ):
            if qc + 1 < NQ:
                load_chunk(qsl, qslp, qT, qc + 1, "q", nc.sync)
                load_chunk(ksl, kslp, kT, qc + 1, "k", nc.sync)
                load_chunk(vsl, vslp, vT, qc + 1, "v", nc.scalar)
                proj_gens.append(projQ_gen(qc + 1))
                proj_gens.append(projK_gen(qc + 1))
                proj_gens.append(projV_gen(qc + 1))
            for p in range(NT):
                if qc == 0 and p == 0:
                    advance(gq, 10**9)
                    advance(gk, 10**9)
                    advance(gv, 10**9)
                attention(p, qc)
            qf = slice(512 * qc, 512 * (qc + 1))
            with nc.allow_low_precision(reason="bf16 recip feeds bf16 matmul"):
                nc.vector.reciprocal(recip[:, qf], sums[:, qf])
            for t in range(NT):
                out_gens.append(norm_gen(t, qc))
            out_gens.append(outproj_gen(qc))
            # issue the DMA-independent norm/outproj work FIRST so the PE has
            # real work while the next chunk's slabs finish streaming in, THEN
            # complete the (DMA-gated) projections before the next chunk.
            if qc + 1 < NQ:
                while out_gens:
                    try:
                        next(out_gens[0])
                    except StopIteration:
                        out_gens.popleft()
            drain_proj()
        drain_all()

    nc.compile()
    return nc


_NC_CACHE = {}


def get_nc():
    if "nc" not in _NC_CACHE:
        _NC_CACHE["nc"] = build_nc()
    return _NC_CACHE["nc"]


def _bf(x):
    return np.ascontiguousarray(np.asarray(x, np.float32)).astype(BF_NP)


def make_in_maps(q, k, v, Wq, bq, Wk, bk, Wv, bv, Wo):
    """Host-side shard prep. Returns list of 8 per-core input dicts."""
    f = np.float32
    q = np.asarray(q, f)
    k = np.asarray(k, f)
    v = np.asarray(v, f)
    Wq, bq = np.asarray(Wq, f), np.asarray(bq, f)
    Wk, bk = np.asarray(Wk, f), np.asarray(bk, f)
    Wv, bv = np.asarray(Wv, f), np.asarray(bv, f)
    Wo = np.asarray(Wo, f)
    E = np.zeros((8, DL), f)
    for h in range(8):
        E[h, 64 * h:64 * (h + 1)] = 1.0
    ident = np.eye(128, dtype=f).astype(BF_NP)
    maskb = (np.triu(np.ones((128, 128), f), k=1) * -3e18).astype(BF_NP)
    in_maps = []
    for c in range(8):
        b, g = c // 2, c % 2
        cs = slice(DL * g, DL * (g + 1))
        in_maps.append(dict(
            qT=_bf(q[b].T),
            kT=_bf(k[b].T),
            vT=_bf(v[b].T),
            Wq_s=_bf(Wq[:, cs]),
            Wk_s=_bf(Wk[:, cs]),
            Wv_s=_bf(Wv[:, cs]),
            Wo_s=_bf(Wo[cs, :]),
            bq_s=np.ascontiguousarray(bq[cs]).reshape(DL, 1),
            bk_s=np.ascontiguousarray(bk[cs]).reshape(DL, 1),
            bv_bc=np.tile(bv[cs][None, :], (128, 1)).astype(f),
            E_in=E.astype(BF_NP),
            ident_in=ident,
            maskb_in=maskb,
        ))
    return in_maps


def unshard(results, bo):
    bo = np.asarray(bo, np.float32)
    out = np.empty((B, S, D), np.float32)
    for b in range(B):
        out[b] = (results[2 * b]["out_partial"].astype(np.float32)
                  + results[2 * b + 1]["out_partial"].astype(np.float32) + bo)
    return out


def kernel(q, k, v, mask, Wq, bq, Wk, bk, Wv, bv, Wo, bo, **_unused):
    nc = get_nc()
    in_maps = make_in_maps(q, k, v, Wq, bq, Wk, bk, Wv, bv, Wo)
    res = run_bass_kernel_spmd(nc, in_maps, core_ids=list(range(8))).results
    return unshard(res, bo)


# revision 57
# speedup vs baseline: 1.0792x; 1.0792x over previous
"""Multi-head attention (B=4, S=2048, D=1024, H=16, causal) on 8 TRN2 cores.

Sharding: core c -> (batch b = c//2, head-group g = c%2 of 8 heads).
Each core computes projections for its 8 heads (column-split Wq/Wk/Wv),
flash-style causal attention, and a partial output projection (row-split Wo).
Host unshard sums the two partials per batch and adds bo.

v2 (all-bf16, fully software-pipelined single pass):
- Everything bf16 on the PE (error budget allows: score errors shrink 8x
  under the 1/sqrt(HD) softmax scale). Halves DMA + SBUF, removes the
  fp32r N<256 4x-cycle penalty, enables FWL weight loads.
- qc-outer schedule: per query chunk, all head pairs run scores->exp->PV
  with a 2-step lag (PV of j-2 issues after scores of j), and projection /
  output-projection matmuls are issued between attention steps as PE
  filler so the PE never idles and HAM stays at 2.4 GHz.
- A+B exp fused into one [128,1024] ACTIVATE per j-block.
- Attention output is written back into the qwT tile (queries for chunk qc
  are dead after their scores), saving 16KB/partition of SBUF.
- Denominators ride the PV matmul as a per-head ones column (M=65).
"""

from collections import deque
from contextlib import ExitStack

import ml_dtypes
import numpy as np

import concourse.bass as bass
import concourse.tile as tile
from concourse import bacc, mybir
from concourse.bass_utils import run_bass_kernel_spmd

F32 = mybir.dt.float32
F16 = mybir.dt.float16
BF = mybir.dt.bfloat16
EXP = mybir.ActivationFunctionType.Exp
COPY = mybir.ActivationFunctionType.Copy
BF_NP = ml_dtypes.bfloat16

B, S, D, H = 4, 2048, 1024, 16
HD = D // H          # 64
DL = D // 2          # 512 local douts per core
NT = DL // 128       # 4 dout tiles / head pairs
NR = S // 128        # 16 key row tiles
NQ = S // 512        # 4 query chunks
NDIN = D // 128      # 8 din tiles
LAG = 3              # scores(j) -> PV(j-LAG) software pipeline depth


def build_nc():
    nc = bacc.Bacc("TRN2", target_bir_lowering=False, debug=False, num_devices=8)

    qT = nc.dram_tensor("qT", [D, S], BF, kind="ExternalInput").ap()
    kT = nc.dram_tensor("kT", [D, S], BF, kind="ExternalInput").ap()
    vT = nc.dram_tensor("vT", [D, S], BF, kind="ExternalInput").ap()
    Wq_s = nc.dram_tensor("Wq_s", [D, DL], BF, kind="ExternalInput").ap()
    Wk_s = nc.dram_tensor("Wk_s", [D, DL], BF, kind="ExternalInput").ap()
    Wv_s = nc.dram_tensor("Wv_s", [D, DL], BF, kind="ExternalInput").ap()
    Wo_s = nc.dram_tensor("Wo_s", [DL, D], BF, kind="ExternalInput").ap()
    bq_s = nc.dram_tensor("bq_s", [DL, 1], F32, kind="ExternalInput").ap()
    bk_s = nc.dram_tensor("bk_s", [DL, 1], F32, kind="ExternalInput").ap()
    bv_bc = nc.dram_tensor("bv_bc", [128, DL], F32, kind="ExternalInput").ap()
    E_in = nc.dram_tensor("E_in", [8, DL], BF, kind="ExternalInput").ap()
    ident_in = nc.dram_tensor("ident_in", [128, 128], BF, kind="ExternalInput").ap()
    maskb_in = nc.dram_tensor("maskb_in", [128, 128], BF, kind="ExternalInput").ap()
    out_p = nc.dram_tensor("out_partial", [S, D], F16, kind="ExternalOutput").ap()

    with tile.TileContext(nc) as tc, ExitStack() as ctx:
        keep = ctx.enter_context(tc.tile_pool(name="keep", bufs=1))
        qslp = ctx.enter_context(tc.tile_pool(name="qsl", bufs=2))
        kslp = ctx.enter_context(tc.tile_pool(name="ksl", bufs=2))
        vslp = ctx.enter_context(tc.tile_pool(name="vsl", bufs=2))
        wtp = ctx.enter_context(tc.tile_pool(name="wt", bufs=1))
        prp = ctx.enter_context(tc.tile_pool(name="probs", bufs=1))
        stgp = ctx.enter_context(tc.tile_pool(name="stg", bufs=2))
        osp = ctx.enter_context(tc.tile_pool(name="osb", bufs=3))
        scp = ctx.enter_context(tc.tile_pool(name="scps", bufs=2, space="PSUM"))
        atp = ctx.enter_context(tc.tile_pool(name="atps", bufs=1, space="PSUM"))
        mip = ctx.enter_context(tc.tile_pool(name="mips", bufs=2, space="PSUM"))

        # ---------------- persistent SBUF ----------------
        # hw[t]: Q^T for head pair t during scores, then overwritten per qc
        # chunk with the (unnormalized) attention output of pair t.
        hw = [keep.tile([128, S], BF, tag=f"hw{t}", name=f"hw{t}") for t in range(NT)]
        kwT = [keep.tile([128, S], BF, tag=f"kwT{t}", name=f"kwT{t}") for t in range(NT)]
        vw = [keep.tile([128, 8 * 65], BF, tag=f"vw{r}", name=f"vw{r}") for r in range(NR)]
        sums = keep.tile([8, S], F32, tag="sums")
        recip = keep.tile([8, S], BF, tag="recip")
        bias_q = keep.tile([128, NT], F32, tag="bias_q")
        bias_k = keep.tile([128, NT], F32, tag="bias_k")
        bv_sb = keep.tile([128, DL], F32, tag="bv_sb")
        E_sb = keep.tile([8, DL], BF, tag="E_sb")
        ident = keep.tile([128, 128], BF, tag="ident")
        maskb = keep.tile([128, 128], BF, tag="maskb")

        # never-written recip rows are multiplied by E's zeros in the
        # broadcast matmul; they must not hold NaN garbage.
        nc.vector.memset(recip[:], 1.0)

        # small/constant loads, spread off the startup critical streams
        for t in range(NT):
            nc.sync.dma_start(bias_q[:, t:t + 1], bq_s[128 * t:128 * (t + 1), :])
            nc.sync.dma_start(bias_k[:, t:t + 1], bk_s[128 * t:128 * (t + 1), :])
        nc.scalar.dma_start(bv_sb[:], bv_bc)
        nc.scalar.dma_start(E_sb[:], E_in)
        nc.scalar.dma_start(ident[:], ident_in)
        nc.scalar.dma_start(maskb[:], maskb_in)

        # per-head ones column in vw (PV emits softmax denominators for free)
        for r in range(NR):
            ones_ap = vw[r][:].rearrange("p (h e) -> p h e", e=65)[:, :, 64:65]
            nc.gpsimd.memset(ones_ap, 1.0)

        # projection weight tiles — DMAs are issued interleaved with the
        # first slab chunks in the schedule section (per-dn pipelining)
        wq_sb = [wtp.tile([128, DL], BF, tag=f"wq{dn}", name=f"wq{dn}")
                 for dn in range(NDIN)]
        wk_sb = [wtp.tile([128, DL], BF, tag=f"wk{dn}", name=f"wk{dn}")
                 for dn in range(NDIN)]
        wv_sb = [wtp.tile([128, DL], BF, tag=f"wv{dn}", name=f"wv{dn}")
                 for dn in range(NDIN)]
        wo_sb = [wtp.tile([128, D], BF, tag=f"wo{t}", name=f"wo{t}")
                 for t in range(NT)]

        # ---------------- chunked slab loads ----------------
        qsl = {}  # qsl[qc][dn] -> [128, 512] bf16 tile of qT
        ksl = {}
        vsl = {}

        def load_chunk(store, pool, src, c, pfx, eng):
            tiles = []
            for dn in range(NDIN):
                t_ = pool.tile([128, 512], BF, tag=f"{pfx}{dn}")
                eng.dma_start(t_[:], src[128 * dn:128 * (dn + 1), 512 * c:512 * (c + 1)])
                tiles.append(t_)
            store[c] = tiles

        # ---------------- filler generators ----------------
        proj_gens = deque()
        out_gens = deque()
        pending_norms = []

        def fill(n):
            done = 0
            while done < n:
                q = proj_gens if proj_gens else out_gens
                if not q:
                    return
                try:
                    next(q[0])
                    done += 1
                except StopIteration:
                    q.popleft()

        def drain_proj():
            while proj_gens:
                try:
                    next(proj_gens[0])
                except StopIteration:
                    proj_gens.popleft()

        def drain_all():
            drain_proj()
            while out_gens:
                try:
                    next(out_gens[0])
                except StopIteration:
                    out_gens.popleft()

        def projQ_gen(qc):
            for t in range(NT):
                ps = mip.tile([128, 512], F32, tag="mx")
                for dn in range(NDIN):
                    if dn % 2 == 0 and dn > 0:
                        yield
                    nc.tensor.matmul(
                        ps[:], wq_sb[dn][:, 128 * t:128 * (t + 1)], qsl[qc][dn][:],
                        start=(dn == 0), stop=(dn == NDIN - 1))
                nc.vector.tensor_scalar_add(
                    hw[t][:, 512 * qc:512 * (qc + 1)], ps[:], bias_q[:, t:t + 1])
                yield

        def projK_gen(rc):
            for t in range(NT):
                ps = mip.tile([128, 512], F32, tag="mx")
                for dn in range(NDIN):
                    if dn % 2 == 0 and dn > 0:
                        yield
                    nc.tensor.matmul(
                        ps[:], wk_sb[dn][:, 128 * t:128 * (t + 1)], ksl[rc][dn][:],
                        start=(dn == 0), stop=(dn == NDIN - 1))
                nc.vector.tensor_scalar_add(
                    kwT[t][:, 512 * rc:512 * (rc + 1)], ps[:], bias_k[:, t:t + 1])
                yield

        def projV_gen(g):
            for r in range(4 * g, 4 * g + 4):
                ps = mip.tile([128, 512], F32, tag="mx")
                for dn in range(NDIN):
                    if dn % 2 == 0 and dn > 0:
                        yield
                    nc.tensor.matmul(
                        ps[:], vsl[g][dn][:, 128 * (r - 4 * g):128 * (r - 4 * g + 1)],
                        wv_sb[dn][:],
                        start=(dn == 0), stop=(dn == NDIN - 1))
                dst3 = vw[r][:].rearrange("p (h e) -> p h e", e=65)[:, :, 0:64]
                nc.vector.tensor_add(
                    dst3, ps[:].rearrange("p (h e) -> p h e", e=64),
                    bv_sb[:].rearrange("p (h e) -> p h e", e=64))
                yield

        def norm_gen(p, qc):
            # broadcast 1/denominator over the pair's 128 dims and rescale
            qf = slice(512 * qc, 512 * (qc + 1))
            bc = mip.tile([128, 512], F32, tag="mx")
            nc.tensor.matmul(bc[:], E_sb[:, 128 * p:128 * (p + 1)],
                             recip[:, qf], start=True, stop=True)
            nc.vector.tensor_mul(hw[p][:, qf], hw[p][:, qf], bc[:])
            yield

        def outproj_gen(qc):
            for rt in range(4 * qc, 4 * qc + 4):
                for nch in range(2):
                    po = mip.tile([128, 512], F32, tag="mx")
                    for t in range(NT):
                        nc.tensor.matmul(
                            po[:], hw[t][:, 128 * rt:128 * (rt + 1)],
                            wo_sb[t][:, 512 * nch:512 * (nch + 1)],
                            start=(t == 0), stop=(t == NT - 1))
                        if t % 2 == 1:
                            yield
                    ob = osp.tile([128, 512], F16, tag="ob")
                    nc.vector.tensor_copy(ob[:], po[:])
                    eng = nc.scalar if rt % 2 else nc.sync
                    eng.dma_start(
                        out_p[128 * rt:128 * (rt + 1), 512 * nch:512 * (nch + 1)], ob[:])

        # ---------------- attention ----------------
        def attention(p, qc):
            jmax = 4 * qc + 3
            atA = atp.tile([65, 512], F32, tag="atA")
            atB = atp.tile([65, 512], F32, tag="atB")
            pend = {}
            for step in range(jmax + 1 + LAG):
                if step <= jmax:
                    j = step
                    off = max(0, 128 * j - 512 * qc)
                    diag = 128 * j >= 512 * qc
                    qs = slice(512 * qc + off, 512 * (qc + 1))
                    sAB = scp.tile([128, 1024], F32, tag="sAB")
                    nc.tensor.matmul(
                        sAB[:, off:512],
                        kwT[p][0:64, 128 * j:128 * (j + 1)], hw[p][0:64, qs],
                        start=True, stop=True, tile_position=(0, 0))
                    nc.tensor.matmul(
                        sAB[:, 512 + off:1024],
                        kwT[p][64:128, 128 * j:128 * (j + 1)], hw[p][64:128, qs],
                        start=True, stop=True, tile_position=(64, 0))
                    pAB = prp.tile([128, 1024], BF, tag=f"p{j % 6}")
                    if off == 0:
                        nc.scalar.activation(pAB[:], sAB[:], EXP, scale=0.125)
                    else:
                        # one strided ACTIVATE covers both halves' live columns
                        s3 = sAB[:].rearrange("p (b c) -> p b c", b=2)[:, :, off:512]
                        p3 = pAB[:].rearrange("p (b c) -> p b c", b=2)[:, :, off:512]
                        nc.scalar.activation(p3, s3, EXP, scale=0.125)
                    if diag:  # causal mask on the diagonal 128-col strip
                        for cb in (off, 512 + off):
                            nc.gpsimd.affine_select(
                                out=pAB[:, cb:cb + 128], in_=pAB[:, cb:cb + 128],
                                channel_multiplier=-1, pattern=[[1, 128]], base=0,
                                compare_op=mybir.AluOpType.is_ge, fill=0.0)
                    pend[j] = (pAB, off)
                jj = step - LAG
                if 0 <= jj:
                    pAB, off = pend.pop(jj)
                    nc.tensor.matmul(
                        atA[0:65, off:512],
                        vw[jj][:, 65 * 2 * p:65 * 2 * p + 65], pAB[:, off:512],
                        start=(jj == 0), stop=(jj == jmax))
                    nc.tensor.matmul(
                        atB[0:65, off:512],
                        vw[jj][:, 65 * (2 * p + 1):65 * (2 * p + 1) + 65],
                        pAB[:, 512 + off:1024],
                        start=(jj == 0), stop=(jj == jmax))
                if step <= jmax:
                    # no fills during the PV drain steps: their DVE consumers
                    # would queue ahead of the epilogue copies below
                    fill(1)
            # epilogue: write attention output over the dead Q columns and
            # stage the denominators (PSUM row 64) out to sums.
            qf = slice(512 * qc, 512 * (qc + 1))
            nc.vector.tensor_copy(hw[p][0:64, qf], atA[0:64, :])
            nc.vector.tensor_copy(hw[p][64:128, qf], atB[0:64, :])
            stgA = stgp.tile([1, 512], F32, tag="stgA")
            stgB = stgp.tile([1, 512], F32, tag="stgB")
            nc.vector.tensor_copy(stgA[:], atA[64:65, :])
            nc.vector.tensor_copy(stgB[:], atB[64:65, :])
            nc.sync.dma_start(sums[2 * p:2 * p + 1, qf], stgA[:])
            nc.sync.dma_start(sums[2 * p + 1:2 * p + 2, qf], stgB[:])
            fill(LAG)

        # ---------------- schedule ----------------
        # startup streams, balanced for ~80GB/s HWDGE queues and the
        # ~170GB/s gpsimd SWDGE bulk path:
        #   gpsimd: wq, wk, wv   sync: q0 + half v0   scalar: k0 + half v0, wo
        for dn in range(NDIN):
            nc.gpsimd.dma_start(wq_sb[dn][:], Wq_s[128 * dn:128 * (dn + 1), :])
        for dn in range(NDIN):
            nc.gpsimd.dma_start(wk_sb[dn][:], Wk_s[128 * dn:128 * (dn + 1), :])
        for dn in range(NDIN):
            nc.gpsimd.dma_start(wv_sb[dn][:], Wv_s[128 * dn:128 * (dn + 1), :])
        q0t, k0t, v0t = [], [], []
        for src, store, pool, pfx in ((qT, q0t, qslp, "q"), (kT, k0t, kslp, "k"),
                                      (vT, v0t, vslp, "v")):
            for dn in range(NDIN):
                t_ = pool.tile([128, 512], BF, tag=f"{pfx}{dn}", name=f"{pfx}0_{dn}")
                eng = nc.sync if dn % 2 == 0 else nc.scalar
                eng.dma_start(t_[:], src[128 * dn:128 * (dn + 1), 0:512])
                store.append(t_)
        qsl[0], ksl[0], vsl[0] = q0t, k0t, v0t
        for t in range(NT):
            nc.scalar.dma_start(wo_sb[t][:], Wo_s[128 * t:128 * (t + 1), :])

        # startup: project pair p's Q/K (and all of V before the first PV),
        # starting attention(p, 0) as soon as its own tiles are ready
        gq, gk, gv = projQ_gen(0), projK_gen(0), projV_gen(0)

        def advance(g, n):
            for _ in range(n):
                try:
                    next(g)
                except StopIteration:
                    return

        for qc in range(NQ):
            if qc + 1 < NQ:
                load_chunk(qsl, qslp, qT, qc + 1, "q", nc.sync)
                load_chunk(ksl, kslp, kT, qc + 1, "k", nc.sync)
                load_chunk(vsl, vslp, vT, qc + 1, "v", nc.scalar)
                proj_gens.append(projQ_gen(qc + 1))
                proj_gens.append(projK_gen(qc + 1))
                proj_gens.append(projV_gen(qc + 1))
            for p in range(NT):
                if qc == 0 and p == 0:
                    advance(gq, 10**9)
                    advance(gk, 10**9)
                    advance(gv, 10**9)
                attention(p, qc)
            drain_proj()  # next chunk's projections must complete before use
            qf = slice(512 * qc, 512 * (qc + 1))
            with nc.allow_low_precision(reason="bf16 recip feeds bf16 matmul"):
                nc.vector.reciprocal(recip[:, qf], sums[:, qf])
            # norm/bc units issue as filler during the NEXT chunk's attention
            # so their recip dependency never head-of-line-stalls the PE
            for t in range(NT):
                out_gens.append(norm_gen(t, qc))
            out_gens.append(outproj_gen(qc))
        drain_all()

    nc.compile()
    return nc


_NC_CACHE = {}


def get_nc():
    if "nc" not in _NC_CACHE:
        _NC_CACHE["nc"] = build_nc()
    return _NC_CACHE["nc"]


def _bf(x):
    return np.ascontiguousarray(np.asarray(x, np.float32)).astype(BF_NP)


def make_in_maps(q, k, v, Wq, bq, Wk, bk, Wv, bv, Wo):
    """Host-side shard prep. Returns list of 8 per-core input dicts."""
    f = np.float32
    q = np.asarray(q, f)
    k = np.asarray(k, f)
    v = np.asarray(v, f)
    Wq, bq = np.asarray(Wq, f), np.asarray(bq, f)
    Wk, bk = np.asarray(Wk, f), np.asarray(bk, f)
    Wv, bv = np.asarray(Wv, f), np.asarray(bv, f)
    Wo = np.asarray(Wo, f)
    E = np.zeros((8, DL), f)
    for h in range(8):
        E[h, 64 * h:64 * (h + 1)] = 1.0
    ident = np.eye(128, dtype=f).astype(BF_NP)
    maskb = (np.triu(np.ones((128, 128), f), k=1) * -3e18).astype(BF_NP)
    in_maps = []
    for c in range(8):
        b, g = c // 2, c % 2
        cs = slice(DL * g, DL * (g + 1))
        in_maps.append(dict(
            qT=_bf(q[b].T),
            kT=_bf(k[b].T),
            vT=_bf(v[b].T),
            Wq_s=_bf(Wq[:, cs]),
            Wk_s=_bf(Wk[:, cs]),
            Wv_s=_bf(Wv[:, cs]),
            Wo_s=_bf(Wo[cs, :]),
            bq_s=np.ascontiguousarray(bq[cs]).reshape(DL, 1),
            bk_s=np.ascontiguousarray(bk[cs]).reshape(DL, 1),
            bv_bc=np.tile(bv[cs][None, :], (128, 1)).astype(f),
            E_in=E.astype(BF_NP),
            ident_in=ident,
            maskb_in=maskb,
        ))
    return in_maps


def unshard(results, bo):
    bo = np.asarray(bo, np.float32)
    out = np.empty((B, S, D), np.float32)
    for b in range(B):
        out[b] = (results[2 * b]["out_partial"].astype(np.float32)
                  + results[2 * b + 1]["out_partial"].astype(np.float32) + bo)
    return out


def kernel(q, k, v, mask, Wq, bq, Wk, bk, Wv, bv, Wo, bo, **_unused):
    nc = get_nc()
    in_maps = make_in_maps(q, k, v, Wq, bq, Wk, bk, Wv, bv, Wo)
    res = run_bass_kernel_spmd(nc, in_maps, core_ids=list(range(8))).results
    return unshard(res, bo)
